# revision 22
# baseline (speedup 1.0000x reference)
"""Trainium2 Bass kernel for nn_MiddleLayerEncoder (gnn_message_passing).

Strategy: shard by CLUSTER across the 8 cores (each core owns 512 whole
clusters and all of their points), so both segment_max reductions are
core-local and no collectives are needed.  Host-side prep sorts points by
cluster and pads every cluster to a canonical per-rank size (multiple of 8,
identical across cores -> single SPMD program).

Both segment-max reductions are fused into PSUM evacuation: DVE
tensor_reduce reads matmul results directly from PSUM through windowed
[p, n, w] views (window-2 for the encoder stage, window-4 for the fc
stage -- safe because every cluster's padded size is a multiple of 8), so
no intermediate copies or strided writes are needed.  The
neigh_enc[cluster] gather is fused into the W1 matmul via per-group
one-hot rows.  Evacuation work is split between Scalar (ACT) and Vector
(DVE); enc1 uses 4 concurrent row-tiled matmuls (K=12).
"""

import numpy as np
import ml_dtypes
from contextlib import ExitStack

import concourse.bass as bass
import concourse.bacc as bacc
import concourse.tile as tile
from concourse import mybir
from concourse.bass_utils import run_bass_kernel_spmd

BF16 = mybir.dt.bfloat16
F32 = mybir.dt.float32
NPBF16 = ml_dtypes.bfloat16

N_CORES = 8
N_PTS = 262144
N_CLUSTERS = 4096
G = 32            # one-hot rows (max clusters overlapping a 1024-col group)
K1 = 3 + 64 + G   # layer-1 contraction: pts(3) + feat(64) + onehot(G) = 99
MINL = 40         # minimum padded points per cluster (multiple of 8)
CHUNK_COLS = 8192
LTILE = 1024      # layer-1 lhsT group width

# engine assignment knobs (per-chunk patterns, tuned against the trace)
L1_DVE_GROUPS = ()        # l1 groups evacuated by DVE instead of ACT


def _chunk_target(idx, remaining):
    """Chunk size schedule: small chunks at both ends so the software
    pipeline fills and drains quickly."""
    if idx == 0:
        return 2048
    if idx == 1:
        return 4096
    if remaining > 14336:
        return CHUNK_COLS
    if remaining > 6144:
        return max(2048, remaining - 6144)
    if remaining > 2048:
        return max(2048, remaining - 2048)
    return 2048


# ---------------------------------------------------------------- planning

def _plan(cluster):
    """Canonical SPMD layout shared by all cores."""
    counts = np.bincount(cluster, minlength=N_CLUSTERS)
    assert counts.min() >= 1, "empty cluster unsupported"
    order = np.argsort(-counts, kind="stable")  # cluster ids, size desc

    # snake-deal into N_CORES bins -> per-core 512 clusters, balanced sizes
    n_ranks = N_CLUSTERS // N_CORES
    cids = np.empty((N_CORES, n_ranks), dtype=np.int64)
    for i, cid in enumerate(order):
        rnd, pos = divmod(i, N_CORES)
        core = pos if rnd % 2 == 0 else N_CORES - 1 - pos
        cids[core, rnd] = cid

    sizes = counts[cids]                      # [cores, ranks]
    L = sizes.max(axis=0)                     # canonical per-rank size
    L = np.maximum((L + 7) // 8 * 8, MINL).astype(np.int64)

    col0 = np.concatenate([[0], np.cumsum(L)])  # rank -> start col
    S = int(col0[-1])

    # chunks: whole clusters, <= target cols, <= 128 clusters; the first
    # chunks are smaller so the software pipeline fills quickly
    chunks = []  # (r0, r1, c0, cc)
    r0 = 0
    while r0 < n_ranks:
        target = _chunk_target(len(chunks), S - int(col0[r0]))
        r1 = r0
        while (
            r1 < n_ranks
            and (col0[r1 + 1] - col0[r0]) <= target
            and (r1 - r0) < 128
        ):
            r1 += 1
        chunks.append((r0, r1, int(col0[r0]), int(col0[r1] - col0[r0])))
        r0 = r1

    col_rank = np.repeat(np.arange(n_ranks), L)   # [S]

    # layer-1 1024-col groups per chunk: (g0, gw, base_rank, gr)
    groups = []
    for (r0c, r1c, c0, cc) in chunks:
        gl = []
        for g0 in range(0, cc, LTILE):
            gw = min(LTILE, cc - g0)
            base = int(col_rank[c0 + g0])
            last = int(col_rank[c0 + g0 + gw - 1])
            nrows = last - base + 1
            assert nrows <= G, f"group spans {nrows} clusters > G={G}"
            gr = min(G, 128 - (base - r0c))
            assert gr >= nrows
            gl.append((g0, gw, base, gr))
        groups.append(gl)

    # size-class runs per chunk (for windowed segment reduces)
    runs = []
    for (r0c, r1c, c0, cc) in chunks:
        rl = []
        i = r0c
        while i < r1c:
            j = i
            while j < r1c and L[j] == L[i]:
                j += 1
            rl.append((i, j, int(L[i])))
            i = j
        runs.append(rl)

    return dict(cids=cids, L=L, col0=col0, S=S, chunks=chunks,
                col_rank=col_rank, groups=groups, runs=runs,
                n_ranks=n_ranks)


def _prep_core(k, plan, rel, feat, sort_idx, bucket0):
    """Per-core input arrays (canonical layout, core-specific data)."""
    L, col0, S = plan["L"], plan["col0"], plan["S"]
    cids = plan["cids"][k]
    n_ranks = plan["n_ranks"]
    n_chunks = len(plan["chunks"])

    slot = np.empty(S, dtype=np.int64)
    for r in range(n_ranks):
        cid = cids[r]
        idx = sort_idx[bucket0[cid]: bucket0[cid + 1]]
        n = idx.shape[0]
        l = int(L[r])
        c0 = int(col0[r])
        slot[c0:c0 + n] = idx
        if l > n:
            slot[c0 + n:c0 + l] = idx[0]

    oh = np.empty(S, dtype=np.int64)
    for ci, (r0c, r1c, c0, cc) in enumerate(plan["chunks"]):
        for (g0, gw, base, gr) in plan["groups"][ci]:
            oh[c0 + g0:c0 + g0 + gw] = \
                plan["col_rank"][c0 + g0:c0 + g0 + gw] - base
    assert oh.min() >= 0 and oh.max() < G

    p = rel[slot]                    # [S, 3] f32
    f = feat[slot]                   # [S, 64] f32

    encT = np.zeros((K1, S), dtype=NPBF16)
    encT[0:3] = p.T.astype(NPBF16)
    encT[3:67] = f.T.astype(NPBF16)
    encT[67 + oh, np.arange(S)] = NPBF16(1.0)

    # pts4c: per chunk [128, 512]; partition block 32j holds quads
    # [512j, 512(j+1)) of the chunk as rows pt*3+coord
    pts4c = np.zeros((128, 512 * n_chunks), dtype=NPBF16)
    for ci, (r0c, r1c, c0, cc) in enumerate(plan["chunks"]):
        Q = cc // 4
        arr = (p[c0:c0 + cc].astype(NPBF16).reshape(Q, 4, 3)
               .transpose(1, 2, 0).reshape(12, Q))
        for j in range((Q + 511) // 512):
            a, b = 512 * j, min(512 * (j + 1), Q)
            pts4c[32 * j:32 * j + 12, 512 * ci: 512 * ci + (b - a)] = arr[:, a:b]
    return {"encT": encT, "pts4c": pts4c}


def _blockdiag(w, times):
    fi, fo = w.shape
    out = np.zeros((fi * times, fo * times), dtype=w.dtype)
    for i in range(times):
        out[i * fi:(i + 1) * fi, i * fo:(i + 1) * fo] = w
    return out


WB_COLS = 1920  # 4x128 mats | W1ab x8 | W1c | enc1 x4 | enc2


def _prep_weights(inp):
    wb = np.zeros((128, WB_COLS), dtype=NPBF16)
    bb = np.zeros((128, 8), dtype=np.float32)
    W1 = inp["W1"]
    wb[:, 0:128] = inp["W2"].astype(NPBF16)
    wb[:, 128:256] = inp["G1"].astype(NPBF16)
    wb[:, 256:384] = inp["G2"][:, 0:128].astype(NPBF16)
    wb[:, 384:512] = inp["G2"][:, 128:256].astype(NPBF16)
    for j in range(8):
        wb[0:67, 512 + 128 * j:512 + 128 * (j + 1)] = W1[0:67].astype(NPBF16)
    wb[0:64, 1536:1664] = W1[67:131].astype(NPBF16)
    e1 = _blockdiag(inp["enc_W1"], 4).astype(NPBF16)   # [12, 128]
    for j in range(4):
        wb[32 * j:32 * j + 12, 1664:1792] = e1
    e2 = _blockdiag(inp["enc_W2"], 2).astype(NPBF16)   # [64, 128]
    wb[0:64, 1792:1920] = e2
    wb[64:128, 1792:1920] = e2
    bb[:, 0] = np.tile(inp["enc_b1"], 4)
    bb[0:64, 1] = inp["enc_b2"]
    bb[:, 2] = inp["b1"]
    bb[:, 3] = inp["b2"]
    bb[:, 4] = inp["gb1"]
    bb[:, 5] = inp["gb2"][0:128]
    bb[:, 6] = inp["gb2"][128:256]
    return {"wblob": np.ascontiguousarray(wb), "bblob": np.ascontiguousarray(bb)}


# ---------------------------------------------------------------- program

def _build(plan):
    S = plan["S"]
    n_chunks = len(plan["chunks"])
    n_ranks = plan["n_ranks"]
    nc = bacc.Bacc(None, target_bir_lowering=False, debug=True)

    encT_d = nc.dram_tensor("encT", [K1, S], BF16, kind="ExternalInput")
    pts4_d = nc.dram_tensor("pts4c", [128, 512 * n_chunks], BF16,
                            kind="ExternalInput")
    wb_d = nc.dram_tensor("wblob", [128, WB_COLS], BF16, kind="ExternalInput")
    bb_d = nc.dram_tensor("bblob", [128, 8], F32, kind="ExternalInput")
    out_d = nc.dram_tensor("out", [256, 512], F32, kind="ExternalOutput")

    RELU = mybir.ActivationFunctionType.Relu
    COPY = mybir.ActivationFunctionType.Copy
    ADD = mybir.AluOpType.add
    MAX = mybir.AluOpType.max
    AXX = mybir.AxisListType.X

    with tile.TileContext(nc) as tc, ExitStack() as ctx:
        consts = ctx.enter_context(tc.tile_pool(name="consts", bufs=1))
        sb_encT = ctx.enter_context(tc.tile_pool(name="sb_encT", bufs=3))
        sb_h1 = ctx.enter_context(tc.tile_pool(name="sb_h1", bufs=2))
        sb_cb = ctx.enter_context(tc.tile_pool(name="sb_cb", bufs=2))
        sb_e1 = ctx.enter_context(tc.tile_pool(name="sb_e1", bufs=2))
        sb_t2 = ctx.enter_context(tc.tile_pool(name="sb_t2", bufs=2))
        sb_sm = ctx.enter_context(tc.tile_pool(name="sb_sm", bufs=2))
        sb_lt = ctx.enter_context(tc.tile_pool(name="sb_lt", bufs=2))
        glob = ctx.enter_context(tc.tile_pool(name="glob", bufs=1))
        ps = ctx.enter_context(tc.tile_pool(name="ps", bufs=4, space="PSUM"))

        wb = consts.tile([128, WB_COLS], BF16, tag="wb")
        nc.sync.dma_start(out=wb[:], in_=wb_d[:])
        bb = consts.tile([128, 8], F32, tag="bb")
        nc.sync.dma_start(out=bb[:], in_=bb_d[:])
        pts_all = consts.tile([128, 512 * n_chunks], BF16, tag="pts_all")
        nc.sync.dma_start(out=pts_all[:], in_=pts4_d[:])
        # preload the Relu activation table during startup DMAs
        tiny = consts.tile([128, 1], F32, tag="tiny")
        nc.scalar.activation(tiny[:], bb[:, 7:8],
                             mybir.ActivationFunctionType.Relu,
                             bias=bb[:, 7:8], scale=1.0)

        fcW2 = wb[:, 0:128]
        G1w = wb[:, 128:256]
        G2aw = wb[:, 256:384]
        G2bw = wb[:, 384:512]
        W1ab8 = wb[0:67, 512:1536]
        W1c = wb[0:64, 1536:1664]
        enc1w = [wb[32 * j:32 * j + 12, 1664:1792] for j in range(4)]
        enc2lo = wb[0:64, 1792:1920]
        enc2hi = wb[64:128, 1792:1920]
        b_enc1 = bb[:, 0:1]
        b_enc2 = bb[0:64, 1:2]
        b1 = bb[:, 2:3]
        b2 = bb[:, 3:4]
        gb1 = bb[:, 4:5]
        gb2a = bb[:, 5:6]
        gb2b = bb[:, 6:7]

        pre_neigh = glob.tile([128, n_ranks], BF16, tag="pre_neigh")
        neighT = glob.tile([64, n_ranks], BF16, tag="neighT")
        T2 = glob.tile([128, n_ranks], BF16, tag="T2")

        # zero both lhsT ring buffers once: rows beyond a group's gr keep
        # stale-but-finite values afterwards (they multiply zero one-hot
        # rows); this only guards against uninitialized-SBUF NaN/Inf
        for _ in range(2):
            ltz = sb_lt.tile([128, 1024], BF16, tag="lt8")
            nc.vector.memset(ltz[:], 0.0)

        def enc_stage(ci):
            (r0c, r1c, c0, cc) = plan["chunks"][ci]
            Q = cc // 4
            encT_t = sb_encT.tile([K1, CHUNK_COLS], BF16, tag="encT")
            nc.sync.dma_start(out=encT_t[:, :cc], in_=encT_d[:, c0:c0 + cc])
            pts_t = pts_all[:, 512 * ci:512 * (ci + 1)]
            h1_t = sb_h1.tile([128, 2048], BF16, tag="h1")
            # enc1: 4 row-tiled concurrent matmuls (K=12 each)
            nsub = (Q + 511) // 512
            pe = []
            for _ in range((nsub + 1) // 2):
                pe_t = ps.tile([128, 1024], F32, tag="ps")
                pe.append(pe_t)
            for j in range(nsub):
                w = min(512, Q - 512 * j)
                nc.tensor.matmul(pe[j // 2][:, 512 * (j % 2):512 * (j % 2) + w],
                                 enc1w[j], pts_t[32 * j:32 * j + 12, :w],
                                 start=True, stop=True,
                                 tile_position=(32 * j, 0))
            for t in range((nsub + 1) // 2):
                w = min(1024, Q - 1024 * t)
                nc.scalar.activation(h1_t[:, 1024 * t:1024 * t + w],
                                     pe[t][:, :w], RELU, bias=b_enc1, scale=1.0)
            # enc2 (K=64 concurrent row-tile pair); psum reduce2 fuses the
            # first two segment-max levels (cluster sizes are multiples of 8)
            a1 = sb_cb.tile([128, 1024], BF16, tag="a1")
            b1v = sb_cb.tile([128, 1024], BF16, tag="b1v")
            cbt = sb_cb.tile([128, 1024], BF16, tag="cbt")
            for t in range((Q + 1023) // 1024):
                w = min(1024, Q - 1024 * t)
                pA = ps.tile([128, 1024], F32, tag="ps")
                pB = ps.tile([128, 1024], F32, tag="ps")
                for u in range(0, w, 512):
                    uw = min(512, w - u)
                    sl = slice(1024 * t + u, 1024 * t + u + uw)
                    nc.tensor.matmul(pA[:, u:u + uw], enc2lo, h1_t[0:64, sl],
                                     start=True, stop=True)
                    nc.tensor.matmul(pB[:, u:u + uw], enc2hi, h1_t[64:128, sl],
                                     start=True, stop=True)
                nc.vector.reduce_max(
                    a1[:, 512 * t:512 * t + w // 2],
                    pA[:, :w].rearrange("p (n w) -> p n w", w=2), axis=AXX)
                nc.vector.reduce_max(
                    b1v[:, 512 * t:512 * t + w // 2],
                    pB[:, :w].rearrange("p (n w) -> p n w", w=2), axis=AXX)
            nc.vector.tensor_max(cbt[:, :Q // 2], a1[:, :Q // 2],
                                 b1v[:, :Q // 2])
            return encT_t, cbt

        def chain_stage(ci, cbt):
            (r0c, r1c, c0, cc) = plan["chunks"][ci]
            nk = r1c - r0c
            # cbt columns are octets (8 points); windows of L/8 per cluster
            for (i, j, l) in plan["runs"][ci]:
                w = l // 8
                o = (int(plan["col0"][i]) - c0) // 8
                n = j - i
                nc.vector.reduce_max(
                    pre_neigh[:, i:j],
                    cbt[:, o:o + n * w].rearrange("p (n w) -> p n w", w=w),
                    axis=AXX)
            fold = sb_sm.tile([64, 128], BF16, tag="fold")
            nc.sync.dma_start(out=fold[:, :nk], in_=pre_neigh[64:128, r0c:r1c])
            mx = sb_sm.tile([64, 128], BF16, tag="mx")
            nc.vector.tensor_max(mx[:, :nk], pre_neigh[0:64, r0c:r1c],
                                 fold[:, :nk])
            nc.scalar.activation(neighT[:, r0c:r1c], mx[:, :nk], RELU,
                                 bias=b_enc2, scale=1.0)

        def M_stage(ci):
            (r0c, r1c, c0, cc) = plan["chunks"][ci]
            nk = r1c - r0c
            pm = ps.tile([128, 1024], F32, tag="ps")
            nc.tensor.matmul(pm[:nk, :128], neighT[:, r0c:r1c], W1c,
                             start=True, stop=True)
            Mc = sb_sm.tile([128, 128], BF16, tag="Mc")
            if nk < 128:
                nc.gpsimd.memset(Mc[:], 0.0)
            nc.scalar.activation(Mc[:nk, :], pm[:nk, :128], COPY)
            ngr = len(plan["groups"][ci])
            lt8 = sb_lt.tile([128, 1024], BF16, tag="lt8")
            nc.sync.dma_start(out=lt8[0:67, :128 * ngr], in_=W1ab8[:, :128 * ngr])
            for gi, (g0, gw, base, gr) in enumerate(plan["groups"][ci]):
                nc.sync.dma_start(out=lt8[67:67 + gr, 128 * gi:128 * gi + 128],
                                  in_=Mc[base - r0c:base - r0c + gr, :])
            return lt8

        def l_stage(ci, encT_t, lt8):
            (r0c, r1c, c0, cc) = plan["chunks"][ci]
            e1 = sb_e1.tile([128, CHUNK_COLS], BF16, tag="e1")
            for gi, (g0, gw, base, gr) in enumerate(plan["groups"][ci]):
                lw = lt8[0:K1, 128 * gi:128 * gi + 128]
                P1 = ps.tile([128, 1024], F32, tag="ps")
                for u in range(0, gw, 512):
                    uw = min(512, gw - u)
                    nc.tensor.matmul(P1[:, u:u + uw], lw,
                                     encT_t[:, g0 + u:g0 + u + uw],
                                     start=True, stop=True)
                dst = e1[:, g0:g0 + gw]
                if gi in L1_DVE_GROUPS:
                    nc.vector.tensor_scalar(dst, P1[:, :gw], b1, 0.0,
                                            op0=ADD, op1=MAX)
                else:
                    nc.scalar.activation(dst, P1[:, :gw], RELU,
                                         bias=b1, scale=1.0)
            # layer 2; psum reduce8 fuses the first three segment-max levels
            # (cluster sizes and offsets are multiples of 8)
            t2p = sb_t2.tile([128, 1024], BF16, tag="t2p")
            for gi, (g0, gw, base, gr) in enumerate(plan["groups"][ci]):
                P2 = ps.tile([128, 1024], F32, tag="ps")
                for u in range(0, gw, 512):
                    uw = min(512, gw - u)
                    nc.tensor.matmul(P2[:, u:u + uw], fcW2,
                                     e1[:, g0 + u:g0 + u + uw],
                                     start=True, stop=True)
                nc.vector.reduce_max(
                    t2p[:, g0 // 8:g0 // 8 + gw // 8],
                    P2[:, :gw].rearrange("p (n w) -> p n w", w=8),
                    axis=AXX)
            # t2p columns are point-octets; windows of L/8 per cluster
            for (i, j, l) in plan["runs"][ci]:
                w = l // 8
                o = (int(plan["col0"][i]) - c0) // 8
                n = j - i
                nc.vector.reduce_max(
                    T2[:, i:j],
                    t2p[:, o:o + n * w].rearrange("p (n w) -> p n w", w=w),
                    axis=AXX)

        # software pipeline: chain(k+1) | enc(k+2) | M(k+1) | l(k) --
        # the seg1 reduce chain is emitted before enc(k+2) so the M matmul
        # (in-order on PE, after enc's matmuls) never waits on it
        enc_res = {}
        lt_of = {}
        enc_res[0] = enc_stage(0)
        chain_stage(0, enc_res[0][1])
        if n_chunks > 1:
            enc_res[1] = enc_stage(1)
        lt_of[0] = M_stage(0)
        for k in range(n_chunks):
            if k + 1 < n_chunks:
                chain_stage(k + 1, enc_res[k + 1][1])
            if k + 2 < n_chunks:
                enc_res[k + 2] = enc_stage(k + 2)
            if k + 1 < n_chunks:
                lt_of[k + 1] = M_stage(k + 1)
            encT_t, _ = enc_res.pop(k)
            l_stage(k, encT_t, lt_of.pop(k))

        # global MLP
        gT = glob.tile([128, n_ranks], BF16, tag="gT")
        nc.scalar.activation(gT[:], T2[:], RELU, bias=b2, scale=1.0)
        pg = ps.tile([128, 1024], F32, tag="ps")
        nc.tensor.matmul(pg[:, :512], G1w, gT[:], start=True, stop=True)
        g1T = glob.tile([128, n_ranks], BF16, tag="g1T")
        nc.scalar.activation(g1T[:], pg[:, :512], RELU, bias=gb1, scale=1.0)
        for half, (wv, bv) in enumerate(((G2aw, gb2a), (G2bw, gb2b))):
            po = ps.tile([128, 1024], F32, tag="ps")
            nc.tensor.matmul(po[:, :512], wv, g1T[:], start=True, stop=True)
            osb = glob.tile([128, 512], F32, tag=f"osb{half}")
            nc.scalar.activation(osb[:], po[:, :512], RELU, bias=bv, scale=1.0)
            nc.sync.dma_start(out=out_d[128 * half:128 * (half + 1), :],
                              in_=osb[:])

    nc.finalize()
    return nc


# ---------------------------------------------------------------- entry

_CACHE = {}


def _run(inputs, trace=False, **spmd_kwargs):
    cluster = np.asarray(inputs["cluster"])
    key = hash(cluster.tobytes())
    if key not in _CACHE:
        plan = _plan(cluster)
        nc = _build(plan)
        _CACHE[key] = (plan, nc)
    plan, nc = _CACHE[key]

    rel = np.asarray(inputs["relative_points"], dtype=np.float32)
    feat = np.asarray(inputs["features"], dtype=np.float32)
    sort_idx = np.argsort(cluster, kind="stable")
    bucket0 = np.concatenate(
        [[0], np.cumsum(np.bincount(cluster, minlength=N_CLUSTERS))]
    )
    wmap = _prep_weights({k: np.asarray(v, dtype=np.float32)
                          for k, v in inputs.items()
                          if k not in ("relative_points", "features", "cluster")})

    in_maps = []
    for k in range(N_CORES):
        m = _prep_core(k, plan, rel, feat, sort_idx, bucket0)
        m.update(wmap)
        in_maps.append(m)

    res = run_bass_kernel_spmd(nc, in_maps, list(range(N_CORES)),
                               trace=trace, **spmd_kwargs)

    out = np.empty((N_CLUSTERS, 256), dtype=np.float32)
    for k in range(N_CORES):
        out[plan["cids"][k]] = res.results[k]["out"].T
    return out, res


def kernel(**inputs):
    return _run(inputs)[0]


# revision 27
# speedup vs baseline: 1.0308x; 1.0308x over previous
"""Trainium2 Bass kernel for nn_MiddleLayerEncoder (gnn_message_passing).

Strategy: shard by CLUSTER across the 8 cores (each core owns 512 whole
clusters and all of their points), so both segment_max reductions are
core-local and no collectives are needed.  Host-side prep sorts points by
cluster and pads every cluster to a canonical per-rank size (multiple of 8,
identical across cores -> single SPMD program).

Both segment-max reductions are fused into PSUM evacuation: DVE
tensor_reduce reads matmul results directly from PSUM through windowed
[p, n, w] views (window-2 for the encoder stage, window-4 for the fc
stage -- safe because every cluster's padded size is a multiple of 8), so
no intermediate copies or strided writes are needed.  The
neigh_enc[cluster] gather is fused into the W1 matmul via per-group
one-hot rows.  Evacuation work is split between Scalar (ACT) and Vector
(DVE); enc1 uses 4 concurrent row-tiled matmuls (K=12).
"""

import numpy as np
import ml_dtypes
from contextlib import ExitStack

import concourse.bass as bass
import concourse.bacc as bacc
import concourse.tile as tile
from concourse import mybir
from concourse.bass_utils import run_bass_kernel_spmd

BF16 = mybir.dt.bfloat16
F32 = mybir.dt.float32
NPBF16 = ml_dtypes.bfloat16

N_CORES = 8
N_PTS = 262144
N_CLUSTERS = 4096
G = 32            # one-hot rows (max clusters overlapping a 1024-col group)
K1 = 3 + 64 + G   # layer-1 contraction: pts(3) + feat(64) + onehot(G) = 99
MINL = 40         # minimum padded points per cluster (multiple of 8)
CHUNK_COLS = 8192
LTILE = 1024      # layer-1 lhsT group width

# engine assignment knobs (per-chunk patterns, tuned against the trace)
L1_DVE_GROUPS = ()        # l1 groups evacuated by DVE instead of ACT


def _chunk_target(idx, remaining):
    """Chunk size schedule: small chunks at both ends so the software
    pipeline fills and drains quickly."""
    if idx == 0:
        return 2048
    if idx == 1:
        return 4096
    if remaining > 12288:
        return CHUNK_COLS
    if remaining > 8192:
        return remaining - 4096
    return remaining


# ---------------------------------------------------------------- planning

def _plan(cluster):
    """Canonical SPMD layout shared by all cores."""
    counts = np.bincount(cluster, minlength=N_CLUSTERS)
    assert counts.min() >= 1, "empty cluster unsupported"
    order = np.argsort(-counts, kind="stable")  # cluster ids, size desc

    # snake-deal into N_CORES bins -> per-core 512 clusters, balanced sizes
    n_ranks = N_CLUSTERS // N_CORES
    cids = np.empty((N_CORES, n_ranks), dtype=np.int64)
    for i, cid in enumerate(order):
        rnd, pos = divmod(i, N_CORES)
        core = pos if rnd % 2 == 0 else N_CORES - 1 - pos
        cids[core, rnd] = cid

    sizes = counts[cids]                      # [cores, ranks]
    L = sizes.max(axis=0)                     # canonical per-rank size
    L = np.maximum((L + 7) // 8 * 8, MINL).astype(np.int64)

    col0 = np.concatenate([[0], np.cumsum(L)])  # rank -> start col
    S = int(col0[-1])

    # chunks: whole clusters, <= target cols, <= 128 clusters; the first
    # chunks are smaller so the software pipeline fills quickly
    chunks = []  # (r0, r1, c0, cc)
    r0 = 0
    while r0 < n_ranks:
        target = _chunk_target(len(chunks), S - int(col0[r0]))
        r1 = r0
        while (
            r1 < n_ranks
            and (col0[r1 + 1] - col0[r0]) <= target
            and (r1 - r0) < 128
        ):
            r1 += 1
        chunks.append((r0, r1, int(col0[r0]), int(col0[r1] - col0[r0])))
        r0 = r1

    col_rank = np.repeat(np.arange(n_ranks), L)   # [S]

    # layer-1 1024-col groups per chunk: (g0, gw, base_rank, gr)
    groups = []
    for (r0c, r1c, c0, cc) in chunks:
        gl = []
        for g0 in range(0, cc, LTILE):
            gw = min(LTILE, cc - g0)
            base = int(col_rank[c0 + g0])
            last = int(col_rank[c0 + g0 + gw - 1])
            nrows = last - base + 1
            assert nrows <= G, f"group spans {nrows} clusters > G={G}"
            gr = min(G, 128 - (base - r0c))
            assert gr >= nrows
            gl.append((g0, gw, base, gr))
        groups.append(gl)

    # size-class runs per chunk (for windowed segment reduces)
    runs = []
    for (r0c, r1c, c0, cc) in chunks:
        rl = []
        i = r0c
        while i < r1c:
            j = i
            while j < r1c and L[j] == L[i]:
                j += 1
            rl.append((i, j, int(L[i])))
            i = j
        runs.append(rl)

    return dict(cids=cids, L=L, col0=col0, S=S, chunks=chunks,
                col_rank=col_rank, groups=groups, runs=runs,
                n_ranks=n_ranks)


def _prep_core(k, plan, rel, feat, sort_idx, bucket0):
    """Per-core input arrays (canonical layout, core-specific data)."""
    L, col0, S = plan["L"], plan["col0"], plan["S"]
    cids = plan["cids"][k]
    n_ranks = plan["n_ranks"]
    n_chunks = len(plan["chunks"])

    slot = np.empty(S, dtype=np.int64)
    for r in range(n_ranks):
        cid = cids[r]
        idx = sort_idx[bucket0[cid]: bucket0[cid + 1]]
        n = idx.shape[0]
        l = int(L[r])
        c0 = int(col0[r])
        slot[c0:c0 + n] = idx
        if l > n:
            slot[c0 + n:c0 + l] = idx[0]

    oh = np.empty(S, dtype=np.int64)
    for ci, (r0c, r1c, c0, cc) in enumerate(plan["chunks"]):
        for (g0, gw, base, gr) in plan["groups"][ci]:
            oh[c0 + g0:c0 + g0 + gw] = \
                plan["col_rank"][c0 + g0:c0 + g0 + gw] - base
    assert oh.min() >= 0 and oh.max() < G

    p = rel[slot]                    # [S, 3] f32
    f = feat[slot]                   # [S, 64] f32

    encT = np.zeros((K1, S), dtype=NPBF16)
    encT[0:3] = p.T.astype(NPBF16)
    encT[3:67] = f.T.astype(NPBF16)
    encT[67 + oh, np.arange(S)] = NPBF16(1.0)

    # pts4c: per chunk [128, 512]; partition block 32j holds quads
    # [512j, 512(j+1)) of the chunk as rows pt*3+coord
    pts4c = np.zeros((128, 512 * n_chunks), dtype=NPBF16)
    for ci, (r0c, r1c, c0, cc) in enumerate(plan["chunks"]):
        Q = cc // 4
        arr = (p[c0:c0 + cc].astype(NPBF16).reshape(Q, 4, 3)
               .transpose(1, 2, 0).reshape(12, Q))
        for j in range((Q + 511) // 512):
            a, b = 512 * j, min(512 * (j + 1), Q)
            pts4c[32 * j:32 * j + 12, 512 * ci: 512 * ci + (b - a)] = arr[:, a:b]
    return {"encT": encT, "pts4c": pts4c}


def _blockdiag(w, times):
    fi, fo = w.shape
    out = np.zeros((fi * times, fo * times), dtype=w.dtype)
    for i in range(times):
        out[i * fi:(i + 1) * fi, i * fo:(i + 1) * fo] = w
    return out


WB_COLS = 1920  # 4x128 mats | W1ab x8 | W1c | enc1 x4 | enc2


def _prep_weights(inp):
    wb = np.zeros((128, WB_COLS), dtype=NPBF16)
    bb = np.zeros((128, 8), dtype=np.float32)
    W1 = inp["W1"]
    wb[:, 0:128] = inp["W2"].astype(NPBF16)
    wb[:, 128:256] = inp["G1"].astype(NPBF16)
    wb[:, 256:384] = inp["G2"][:, 0:128].astype(NPBF16)
    wb[:, 384:512] = inp["G2"][:, 128:256].astype(NPBF16)
    for j in range(8):
        wb[0:67, 512 + 128 * j:512 + 128 * (j + 1)] = W1[0:67].astype(NPBF16)
    wb[0:64, 1536:1664] = W1[67:131].astype(NPBF16)
    e1 = _blockdiag(inp["enc_W1"], 4).astype(NPBF16)   # [12, 128]
    for j in range(4):
        wb[32 * j:32 * j + 12, 1664:1792] = e1
    e2 = _blockdiag(inp["enc_W2"], 2).astype(NPBF16)   # [64, 128]
    wb[0:64, 1792:1920] = e2
    wb[64:128, 1792:1920] = e2
    bb[:, 0] = np.tile(inp["enc_b1"], 4)
    bb[0:64, 1] = inp["enc_b2"]
    bb[:, 2] = inp["b1"]
    bb[:, 3] = inp["b2"]
    bb[:, 4] = inp["gb1"]
    bb[:, 5] = inp["gb2"][0:128]
    bb[:, 6] = inp["gb2"][128:256]
    return {"wblob": np.ascontiguousarray(wb), "bblob": np.ascontiguousarray(bb)}


# ---------------------------------------------------------------- program

def _build(plan):
    S = plan["S"]
    n_chunks = len(plan["chunks"])
    n_ranks = plan["n_ranks"]
    nc = bacc.Bacc(None, target_bir_lowering=False, debug=True)

    encT_d = nc.dram_tensor("encT", [K1, S], BF16, kind="ExternalInput")
    pts4_d = nc.dram_tensor("pts4c", [128, 512 * n_chunks], BF16,
                            kind="ExternalInput")
    wb_d = nc.dram_tensor("wblob", [128, WB_COLS], BF16, kind="ExternalInput")
    bb_d = nc.dram_tensor("bblob", [128, 8], F32, kind="ExternalInput")
    out_d = nc.dram_tensor("out", [256, 512], F32, kind="ExternalOutput")

    RELU = mybir.ActivationFunctionType.Relu
    COPY = mybir.ActivationFunctionType.Copy
    ADD = mybir.AluOpType.add
    MAX = mybir.AluOpType.max
    AXX = mybir.AxisListType.X

    with tile.TileContext(nc) as tc, ExitStack() as ctx:
        consts = ctx.enter_context(tc.tile_pool(name="consts", bufs=1))
        sb_encT = ctx.enter_context(tc.tile_pool(name="sb_encT", bufs=3))
        sb_h1 = ctx.enter_context(tc.tile_pool(name="sb_h1", bufs=2))
        sb_cb = ctx.enter_context(tc.tile_pool(name="sb_cb", bufs=2))
        sb_e1 = ctx.enter_context(tc.tile_pool(name="sb_e1", bufs=2))
        sb_t2 = ctx.enter_context(tc.tile_pool(name="sb_t2", bufs=2))
        sb_sm = ctx.enter_context(tc.tile_pool(name="sb_sm", bufs=2))
        sb_lt = ctx.enter_context(tc.tile_pool(name="sb_lt", bufs=2))
        glob = ctx.enter_context(tc.tile_pool(name="glob", bufs=1))
        ps = ctx.enter_context(tc.tile_pool(name="ps", bufs=4, space="PSUM"))

        wb = consts.tile([128, WB_COLS], BF16, tag="wb")
        nc.sync.dma_start(out=wb[:], in_=wb_d[:])
        bb = consts.tile([128, 8], F32, tag="bb")
        nc.sync.dma_start(out=bb[:], in_=bb_d[:])
        pts_all = consts.tile([128, 512 * n_chunks], BF16, tag="pts_all")
        nc.sync.dma_start(out=pts_all[:], in_=pts4_d[:])
        # preload the Relu activation table during startup DMAs
        tiny = consts.tile([128, 1], F32, tag="tiny")
        nc.scalar.activation(tiny[:], bb[:, 7:8],
                             mybir.ActivationFunctionType.Relu,
                             bias=bb[:, 7:8], scale=1.0)

        fcW2 = wb[:, 0:128]
        G1w = wb[:, 128:256]
        G2aw = wb[:, 256:384]
        G2bw = wb[:, 384:512]
        W1ab8 = wb[0:67, 512:1536]
        W1c = wb[0:64, 1536:1664]
        enc1w = [wb[32 * j:32 * j + 12, 1664:1792] for j in range(4)]
        enc2lo = wb[0:64, 1792:1920]
        enc2hi = wb[64:128, 1792:1920]
        b_enc1 = bb[:, 0:1]
        b_enc2 = bb[0:64, 1:2]
        b1 = bb[:, 2:3]
        b2 = bb[:, 3:4]
        gb1 = bb[:, 4:5]
        gb2a = bb[:, 5:6]
        gb2b = bb[:, 6:7]

        pre_neigh = glob.tile([128, n_ranks], BF16, tag="pre_neigh")
        neighT = glob.tile([64, n_ranks], BF16, tag="neighT")
        T2 = glob.tile([128, n_ranks], BF16, tag="T2")

        # zero both lhsT ring buffers once: rows beyond a group's gr keep
        # stale-but-finite values afterwards (they multiply zero one-hot
        # rows); this only guards against uninitialized-SBUF NaN/Inf
        for _ in range(2):
            ltz = sb_lt.tile([128, 1024], BF16, tag="lt8")
            nc.gpsimd.memset(ltz[:], 0.0)

        def enc_stage(ci):
            (r0c, r1c, c0, cc) = plan["chunks"][ci]
            Q = cc // 4
            encT_t = sb_encT.tile([K1, CHUNK_COLS], BF16, tag="encT")
            nc.sync.dma_start(out=encT_t[:, :cc], in_=encT_d[:, c0:c0 + cc])
            pts_t = pts_all[:, 512 * ci:512 * (ci + 1)]
            h1_t = sb_h1.tile([128, 2048], BF16, tag="h1")
            # enc1: 4 row-tiled concurrent matmuls (K=12 each)
            nsub = (Q + 511) // 512
            pe = []
            for _ in range((nsub + 1) // 2):
                pe_t = ps.tile([128, 1024], F32, tag="ps")
                pe.append(pe_t)
            for j in range(nsub):
                w = min(512, Q - 512 * j)
                nc.tensor.matmul(pe[j // 2][:, 512 * (j % 2):512 * (j % 2) + w],
                                 enc1w[j], pts_t[32 * j:32 * j + 12, :w],
                                 start=True, stop=True,
                                 tile_position=(32 * j, 0))
            for t in range((nsub + 1) // 2):
                w = min(1024, Q - 1024 * t)
                nc.scalar.activation(h1_t[:, 1024 * t:1024 * t + w],
                                     pe[t][:, :w], RELU, bias=b_enc1, scale=1.0)
            # enc2 (K=64 concurrent row-tile pair); psum reduce2 fuses the
            # first two segment-max levels (cluster sizes are multiples of 8)
            a1 = sb_cb.tile([128, 1024], BF16, tag="a1")
            b1v = sb_cb.tile([128, 1024], BF16, tag="b1v")
            cbt = sb_cb.tile([128, 1024], BF16, tag="cbt")
            for t in range((Q + 1023) // 1024):
                w = min(1024, Q - 1024 * t)
                pA = ps.tile([128, 1024], F32, tag="ps")
                pB = ps.tile([128, 1024], F32, tag="ps")
                for u in range(0, w, 512):
                    uw = min(512, w - u)
                    sl = slice(1024 * t + u, 1024 * t + u + uw)
                    nc.tensor.matmul(pA[:, u:u + uw], enc2lo, h1_t[0:64, sl],
                                     start=True, stop=True)
                    nc.tensor.matmul(pB[:, u:u + uw], enc2hi, h1_t[64:128, sl],
                                     start=True, stop=True)
                nc.vector.reduce_max(
                    a1[:, 512 * t:512 * t + w // 2],
                    pA[:, :w].rearrange("p (n w) -> p n w", w=2), axis=AXX)
                nc.vector.reduce_max(
                    b1v[:, 512 * t:512 * t + w // 2],
                    pB[:, :w].rearrange("p (n w) -> p n w", w=2), axis=AXX)
            nc.vector.tensor_max(cbt[:, :Q // 2], a1[:, :Q // 2],
                                 b1v[:, :Q // 2])
            return encT_t, cbt

        def chain_stage(ci, cbt):
            (r0c, r1c, c0, cc) = plan["chunks"][ci]
            nk = r1c - r0c
            # cbt columns are octets (8 points); windows of L/8 per cluster
            for (i, j, l) in plan["runs"][ci]:
                w = l // 8
                o = (int(plan["col0"][i]) - c0) // 8
                n = j - i
                nc.vector.reduce_max(
                    pre_neigh[:, i:j],
                    cbt[:, o:o + n * w].rearrange("p (n w) -> p n w", w=w),
                    axis=AXX)
            fold = sb_sm.tile([64, 128], BF16, tag="fold")
            nc.sync.dma_start(out=fold[:, :nk], in_=pre_neigh[64:128, r0c:r1c])
            mx = sb_sm.tile([64, 128], BF16, tag="mx")
            nc.vector.tensor_max(mx[:, :nk], pre_neigh[0:64, r0c:r1c],
                                 fold[:, :nk])
            nc.scalar.activation(neighT[:, r0c:r1c], mx[:, :nk], RELU,
                                 bias=b_enc2, scale=1.0)

        def M_stage(ci):
            (r0c, r1c, c0, cc) = plan["chunks"][ci]
            nk = r1c - r0c
            pm = ps.tile([128, 1024], F32, tag="ps")
            nc.tensor.matmul(pm[:nk, :128], neighT[:, r0c:r1c], W1c,
                             start=True, stop=True)
            Mc = sb_sm.tile([128, 128], BF16, tag="Mc")
            if nk < 128:
                nc.gpsimd.memset(Mc[:], 0.0)
            nc.scalar.activation(Mc[:nk, :], pm[:nk, :128], COPY)
            ngr = len(plan["groups"][ci])
            lt8 = sb_lt.tile([128, 1024], BF16, tag="lt8")
            nc.sync.dma_start(out=lt8[0:67, :128 * ngr], in_=W1ab8[:, :128 * ngr])
            for gi, (g0, gw, base, gr) in enumerate(plan["groups"][ci]):
                eng = nc.scalar if gi % 2 == 0 else nc.sync
                eng.dma_start(out=lt8[67:67 + gr, 128 * gi:128 * gi + 128],
                              in_=Mc[base - r0c:base - r0c + gr, :])
            return lt8

        def l_stage(ci, encT_t, lt8):
            (r0c, r1c, c0, cc) = plan["chunks"][ci]
            e1 = sb_e1.tile([128, CHUNK_COLS], BF16, tag="e1")
            for gi, (g0, gw, base, gr) in enumerate(plan["groups"][ci]):
                lw = lt8[0:K1, 128 * gi:128 * gi + 128]
                P1 = ps.tile([128, 1024], F32, tag="ps")
                for u in range(0, gw, 512):
                    uw = min(512, gw - u)
                    nc.tensor.matmul(P1[:, u:u + uw], lw,
                                     encT_t[:, g0 + u:g0 + u + uw],
                                     start=True, stop=True)
                dst = e1[:, g0:g0 + gw]
                if gi in L1_DVE_GROUPS:
                    nc.vector.tensor_scalar(dst, P1[:, :gw], b1, 0.0,
                                            op0=ADD, op1=MAX)
                else:
                    nc.scalar.activation(dst, P1[:, :gw], RELU,
                                         bias=b1, scale=1.0)
            # layer 2; psum reduce8 fuses the first three segment-max levels
            # (cluster sizes and offsets are multiples of 8)
            t2p = sb_t2.tile([128, 1024], BF16, tag="t2p")
            for gi, (g0, gw, base, gr) in enumerate(plan["groups"][ci]):
                P2 = ps.tile([128, 1024], F32, tag="ps")
                for u in range(0, gw, 512):
                    uw = min(512, gw - u)
                    nc.tensor.matmul(P2[:, u:u + uw], fcW2,
                                     e1[:, g0 + u:g0 + u + uw],
                                     start=True, stop=True)
                nc.vector.reduce_max(
                    t2p[:, g0 // 8:g0 // 8 + gw // 8],
                    P2[:, :gw].rearrange("p (n w) -> p n w", w=8),
                    axis=AXX)
            # t2p columns are point-octets; windows of L/8 per cluster
            for (i, j, l) in plan["runs"][ci]:
                w = l // 8
                o = (int(plan["col0"][i]) - c0) // 8
                n = j - i
                nc.vector.reduce_max(
                    T2[:, i:j],
                    t2p[:, o:o + n * w].rearrange("p (n w) -> p n w", w=w),
                    axis=AXX)

        # software pipeline: chain(k+1) | enc(k+2) | M(k+1) | l(k) --
        # the seg1 reduce chain is emitted before enc(k+2) so the M matmul
        # (in-order on PE, after enc's matmuls) never waits on it
        enc_res = {}
        lt_of = {}
        enc_res[0] = enc_stage(0)
        chain_stage(0, enc_res[0][1])
        if n_chunks > 1:
            enc_res[1] = enc_stage(1)
        lt_of[0] = M_stage(0)
        for k in range(n_chunks):
            if k + 1 < n_chunks:
                chain_stage(k + 1, enc_res[k + 1][1])
            if k + 2 < n_chunks:
                enc_res[k + 2] = enc_stage(k + 2)
            if k + 1 < n_chunks:
                lt_of[k + 1] = M_stage(k + 1)
            encT_t, _ = enc_res.pop(k)
            l_stage(k, encT_t, lt_of.pop(k))

        # global MLP
        gT = glob.tile([128, n_ranks], BF16, tag="gT")
        nc.scalar.activation(gT[:], T2[:], RELU, bias=b2, scale=1.0)
        pg = ps.tile([128, 1024], F32, tag="ps")
        nc.tensor.matmul(pg[:, :512], G1w, gT[:], start=True, stop=True)
        g1T = glob.tile([128, n_ranks], BF16, tag="g1T")
        nc.scalar.activation(g1T[:], pg[:, :512], RELU, bias=gb1, scale=1.0)
        for half, (wv, bv) in enumerate(((G2aw, gb2a), (G2bw, gb2b))):
            po = ps.tile([128, 1024], F32, tag="ps")
            nc.tensor.matmul(po[:, :512], wv, g1T[:], start=True, stop=True)
            osb = glob.tile([128, 512], F32, tag=f"osb{half}")
            nc.scalar.activation(osb[:], po[:, :512], RELU, bias=bv, scale=1.0)
            nc.sync.dma_start(out=out_d[128 * half:128 * (half + 1), :],
                              in_=osb[:])

    nc.finalize()
    return nc


# ---------------------------------------------------------------- entry

_CACHE = {}


def _run(inputs, trace=False, **spmd_kwargs):
    cluster = np.asarray(inputs["cluster"])
    key = hash(cluster.tobytes())
    if key not in _CACHE:
        plan = _plan(cluster)
        nc = _build(plan)
        _CACHE[key] = (plan, nc)
    plan, nc = _CACHE[key]

    rel = np.asarray(inputs["relative_points"], dtype=np.float32)
    feat = np.asarray(inputs["features"], dtype=np.float32)
    sort_idx = np.argsort(cluster, kind="stable")
    bucket0 = np.concatenate(
        [[0], np.cumsum(np.bincount(cluster, minlength=N_CLUSTERS))]
    )
    wmap = _prep_weights({k: np.asarray(v, dtype=np.float32)
                          for k, v in inputs.items()
                          if k not in ("relative_points", "features", "cluster")})

    in_maps = []
    for k in range(N_CORES):
        m = _prep_core(k, plan, rel, feat, sort_idx, bucket0)
        m.update(wmap)
        in_maps.append(m)

    res = run_bass_kernel_spmd(nc, in_maps, list(range(N_CORES)),
                               trace=trace, **spmd_kwargs)

    out = np.empty((N_CLUSTERS, 256), dtype=np.float32)
    for k in range(N_CORES):
        out[plan["cids"][k]] = res.results[k]["out"].T
    return out, res


def kernel(**inputs):
    return _run(inputs)[0]
